# revision 33
# baseline (speedup 1.0000x reference)
"""Multi-head attention Trainium2 kernel (8 NeuronCores, SPMD).

Sharding: core = (batch b in {0,1}, head-group g in {0..3}); each core computes
4 heads' attention for one batch plus its slice of the output projection
(Wo rows for its heads); host sums the 4 per-batch partials (the "all-reduce")
and adds bo.

Device math (per core, heads grouped in pairs that share the 128-partition dim):
  xT = x[b].T                               [D=1024, S=2048]   (host-transposed)
  Q^T/K^T/V^T pair tiles = W_pair.T @ xT    [128 (=2*64 dh), 2048]
  scores^T[sk,sq] = (K^T slice).T @ Q^T     contraction dh=64
  E^T = exp(0.125 * scores^T)               (softmax max-subtract skipped: logits ~N(0,1))
  out'[dh+1, sq] = [V|1].T @ E^T            row 64 = softmax denominators (free)
  out^T = out'[0:64] * (1/out'[64]) bcast   (recip = exp(-ln(s)), one ACT table set)
  y[sq, dout] = concat^T.T @ Wo_local       accumulated over the 2 head pairs

Matmul operands use float32r (fp32 rounded to 11-bit mantissa, full PE rate
at N>=256); accumulation fp32 in PSUM; host pre-rounds the f32r inputs. Emission order interleaves pair-23 QKV
with pair-01 attention so ScalarE (the exp bottleneck) starts early.
"""

import sys

if "/opt/trn_rl_repo" not in sys.path:
    sys.path.insert(0, "/opt/trn_rl_repo")

import numpy as np

import concourse.bass as bass
import concourse.tile as tile
from concourse import bacc, mybir
from concourse.bass_utils import run_bass_kernel_spmd

F32 = mybir.dt.float32
DT = mybir.dt.float32r  # matmul operand dtype

B, S, D, H, DH = 2, 2048, 1024, 16, 64
NCORES = 8
HG = H // 4            # 4 head-groups
ST = S // 128          # 16 s-tiles
KT = D // 128          # 8 d-tiles
SCALE = 1.0 / np.sqrt(DH)

_CACHE: dict = {}


def round_f32r(a):
    """Round fp32 to float32r (11-bit mantissa, round-to-nearest) — the
    operand format the PE consumes at full rate; HW-verified bit pattern."""
    u = np.ascontiguousarray(a, dtype=np.float32).view(np.uint32)
    u = (u + np.uint32(0x7FF) + ((u >> np.uint32(12)) & np.uint32(1))) \
        & np.uint32(0xFFFFF000)
    return u.view(np.float32)


def _patch_act_tables():
    """Restrict ACT table choice to the set containing both Exp and Ln so the
    softmax exp stream and the ln/exp reciprocal share one table (no ~2.7us
    ACT_TABLE_LOAD switches)."""
    import concourse.hw_specs as hw_specs

    if getattr(hw_specs, "_mha_patched", False):
        return
    orig = hw_specs.get_activation_tables

    def patched(arch, *a, **k):
        tabs = orig(arch, *a, **k)
        if "natural_log_exp_and_others" not in tabs:
            return tabs
        # act_func_set_id is positional (index into act_info.json) — keep all
        # entries in order, just hide Exp/Ln from the other sets so the
        # selector always lands on the joint set.
        out = {}
        for n, fns in tabs.items():
            if n != "natural_log_exp_and_others":
                fns = {f for f in fns
                       if str(f).split(".")[-1] not in ("Exp", "Ln")}
            out[n] = set(fns)
        return out

    hw_specs.get_activation_tables = patched
    bacc.get_activation_tables = patched
    hw_specs._mha_patched = True


def build_nc(dbg=False, repeat=1):
    _patch_act_tables()
    nc = bacc.Bacc(None, target_bir_lowering=False, debug=False)

    xT = nc.dram_tensor("xT", [D, S], DT, kind="ExternalInput")
    wqkv = nc.dram_tensor("wqkv", [2, 3, D, 128], DT, kind="ExternalInput")
    bqkv = nc.dram_tensor("bqkv", [2, 3, 128], F32, kind="ExternalInput")
    wo = nc.dram_tensor("wo", [2, 128, D], DT, kind="ExternalInput")
    ident_in = nc.dram_tensor("ident", [128, 128], DT, kind="ExternalInput")
    y = nc.dram_tensor("y", [S, D], F32, kind="ExternalOutput")
    if dbg:
        dbg_t = {n: nc.dram_tensor(f"dbg_{n}", [128, S], F32,
                                   kind="ExternalOutput")
                 for n in ("xt0", "qt0", "kt0", "vt0", "vp0", "e0", "cat0",
                           "cs0")}

    Exp = mybir.ActivationFunctionType.Exp
    Ln = mybir.ActivationFunctionType.Ln

    from contextlib import ExitStack

    with tile.TileContext(nc) as tc:
        with ExitStack() as ctx:
            stage_p = ctx.enter_context(tc.tile_pool(name="stage", bufs=1))
            xt_p = ctx.enter_context(tc.tile_pool(name="xt", bufs=8))
            qkvp_p = ctx.enter_context(tc.tile_pool(name="qkvp", bufs=5))
            vpp_p = ctx.enter_context(tc.tile_pool(name="vpp", bufs=4))
            wp_p = ctx.enter_context(tc.tile_pool(name="wp", bufs=2))
            wop_p = ctx.enter_context(tc.tile_pool(name="wop", bufs=2))
            ep_p = ctx.enter_context(tc.tile_pool(name="ep", bufs=6))
            rin_p = ctx.enter_context(tc.tile_pool(name="rinp", bufs=1))
            rb_p = ctx.enter_context(tc.tile_pool(name="rbp", bufs=1))
            ntmp_p = ctx.enter_context(tc.tile_pool(name="ntmp", bufs=1))
            cat_p = ctx.enter_context(tc.tile_pool(name="catp", bufs=2))
            fo_p = ctx.enter_context(tc.tile_pool(name="fop", bufs=2))
            bias_p = ctx.enter_context(tc.tile_pool(name="bias", bufs=1))
            id_p = ctx.enter_context(tc.tile_pool(name="idp", bufs=1))
            dram_p = ctx.enter_context(tc.tile_pool(name="drs", bufs=2, space="DRAM"))
            psA = ctx.enter_context(tc.tile_pool(name="psA", bufs=2, space="PSUM"))
            psB = ctx.enter_context(tc.tile_pool(name="psB", bufs=1, space="PSUM"))

            def load_rounded(dst_dt, src_ap):
                """DMA from a float32r-typed DRAM tensor into a float32r
                tile (HW consumes the fp32 bits in f32r matmul mode)."""
                nc.sync.dma_start(out=dst_dt, in_=src_ap)

            # ---- small loads first (so they aren't queued behind 8MB of xT) ----
            ident = id_p.tile([128, 128], DT, name="ident")
            load_rounded(ident, ident_in[:, :])

            bias = bias_p.tile([128, 6], F32, name="bias")
            for p in range(2):
                for j in range(3):
                    src = bqkv[p, j, :]
                    src2 = bass.AP(tensor=src.tensor, offset=src.offset,
                                   ap=list(src.ap) + [[1, 1]])
                    nc.sync.dma_start(out=bias[:, 3 * p + j:3 * p + j + 1],
                                      in_=src2)

            qt, kt, vt = [None, None], [None, None], [None, None]
            vpp = [None] * 4
            cat = []
            for p in range(2):
                c_t = cat_p.tile([128, S], DT, name=f"cat{p}", tag="cat")
                cat.append(c_t)

            def load_wset(p, j):
                w = wp_p.tile([128, D], DT, name="wset", tag="w")
                nc.sync.dma_start(
                    out=w.rearrange("dp (k m) -> dp k m", k=KT),
                    in_=wqkv[p, j].rearrange("(k dp) m -> dp k m", dp=128))
                dst = qkvp_p.tile([128, S], DT, name=f"qkv{p}{j}", tag="qkv")
                [qt, kt, vt][j][p] = dst
                return w, dst

            def emit_qkv_set(p, j, use_psB=False):
                """One stationary set: produces the (pair p, proj j) pair tile.

                use_psB: accumulate both 1024-chunks in one psB [128,2048] tile
                (for sets emitted while psA is owned by the scores pipeline)."""
                w, dst = load_wset(p, j)
                if use_psB:
                    acc2 = psB.tile([128, 2048], F32, name="qkvaccB", tag="psB")
                for cp in range(2):
                    acc = (acc2[:, 1024 * cp:1024 * (cp + 1)] if use_psB else
                           psA.tile([128, 1024], F32, name="qkvacc", tag="psA"))
                    for k in range(KT):
                        for c in range(2):
                            nc.tensor.matmul(
                                acc[:, 512 * c:512 * (c + 1)],
                                w[:, 128 * k:128 * (k + 1)],
                                xt[k][:, 1024 * cp + 512 * c:
                                       1024 * cp + 512 * (c + 1)],
                                start=(k == 0), stop=(k == KT - 1))
                    nc.vector.tensor_scalar_add(
                        dst[:, 1024 * cp:1024 * (cp + 1)], acc,
                        bias[:, 3 * p + j:3 * p + j + 1])

            def emit_qk_chase(p, wq, dq, wk, dk):
                """q and k sets together, interleaved per d-tile so both chase
                the x^T DMA stream concurrently (q in the 2 psA slots, k in one
                psB tile). w tiles were loaded before the x^T stream so their
                rounding copies aren't queued behind the x^T ones."""
                accq = [psA.tile([128, 1024], F32, name="qkvacc", tag="psA")
                        for _ in range(2)]
                acck = psB.tile([128, 2048], F32, name="qkvaccB", tag="psB")
                for k in range(KT):
                    for w_, acc_of in ((wq, lambda cp: accq[cp]),
                                       (wk, lambda cp: acck[:, 1024 * cp:
                                                            1024 * (cp + 1)])):
                        for cp in range(2):
                            for c in range(2):
                                nc.tensor.matmul(
                                    acc_of(cp)[:, 512 * c:512 * (c + 1)],
                                    w_[:, 128 * k:128 * (k + 1)],
                                    xt[k][:, 1024 * cp + 512 * c:
                                           1024 * cp + 512 * (c + 1)],
                                    start=(k == 0), stop=(k == KT - 1))
                for cp in range(2):
                    nc.vector.tensor_scalar_add(
                        dq[:, 1024 * cp:1024 * (cp + 1)], accq[cp],
                        bias[:, 3 * p + 0:3 * p + 1])
                    nc.vector.tensor_scalar_add(
                        dk[:, 1024 * cp:1024 * (cp + 1)],
                        acck[:, 1024 * cp:1024 * (cp + 1)],
                        bias[:, 3 * p + 1:3 * p + 2])

            def emit_vprime_pair(p):
                """V' = [V | ones] in [sk, dh+1] layout for both heads of a
                pair: one [128,128] PE transpose per s-tile covers both heads.
                Copies are chunked (4 s-tiles each) so the first AV matmuls can
                start before the whole pair is evacuated."""
                tr = psB.tile([128, 2048], F32, name="trps", tag="psB")
                vps = []
                for hh in range(2):
                    vp = vpp_p.tile([128, ST, 65], DT, name=f"vp{2*p+hh}",
                                    tag="vp")
                    vpp[2 * p + hh] = vp
                    vps.append(vp)
                tr3 = tr.rearrange("q (t c) -> q t c", c=128)
                for ck in range(0, ST, 4):
                    for sk in range(ck, ck + 4):
                        nc.tensor.transpose(
                            tr[:, 128 * sk:128 * (sk + 1)].bitcast(DT),
                            vt[p][:, 128 * sk:128 * (sk + 1)],
                            ident)
                    for hh in range(2):
                        nc.vector.tensor_copy(
                            vps[hh][:, ck:ck + 4, 0:64],
                            tr3[:, ck:ck + 4, 64 * hh:64 * (hh + 1)])
                for hh in range(2):
                    # ones column: x*0 + 1 (memset can't produce float32r)
                    nc.vector.tensor_scalar(
                        out=vps[hh][:, :, 64:65], in0=vps[hh][:, :, 0:1],
                        scalar1=0.0, scalar2=1.0,
                        op0=mybir.AluOpType.mult, op1=mybir.AluOpType.add)

            def emit_attention(p, prefix=None, half_hooks=(None, None)):
                for half in range(2):
                    if half_hooks[half] is not None:
                        half_hooks[half]()

                    def scores_exp(sk):
                        etiles = []
                        for hh in range(2):
                            sc = psA.tile([128, 1024], F32, name="scps", tag="psA")
                            for c in range(2):
                                nc.tensor.matmul(
                                    sc[:, 512 * c:512 * (c + 1)],
                                    kt[p][64 * hh:64 * (hh + 1),
                                          128 * sk:128 * (sk + 1)],
                                    qt[p][64 * hh:64 * (hh + 1),
                                          1024 * half + 512 * c:
                                          1024 * half + 512 * (c + 1)],
                                    start=True, stop=True)
                            e = ep_p.tile([128, 1024], DT, name="et", tag="e")
                            nc.scalar.activation(out=e, in_=sc, func=Exp,
                                                 scale=float(SCALE))
                            etiles.append(e)
                        return etiles

                    def avs(av, sk, etiles):
                        for hh in range(2):
                            for c in range(2):
                                nc.tensor.matmul(
                                    av[0:65, 1024 * hh + 512 * c:
                                       1024 * hh + 512 * (c + 1)],
                                    vpp[2 * p + hh][:, sk, :],
                                    etiles[hh][:, 512 * c:512 * (c + 1)],
                                    start=(sk == 0), stop=(sk == ST - 1))

                    if half == 0 and prefix is not None:
                        # first exps flow while the v-set + V' generation runs
                        pre = [scores_exp(0), scores_exp(1)]
                        prefix()
                        av = psB.tile([128, 2048], F32, name="avps", tag="psB")
                        for sk in range(2):
                            avs(av, sk, pre[sk])
                        start_sk = 2
                    else:
                        av = psB.tile([128, 2048], F32, name="avps", tag="psB")
                        start_sk = 0
                    for sk in range(start_sk, ST):
                        avs(av, sk, scores_exp(sk))

                    # evacuate av (incl. denominator row 64) to SBUF in one
                    # copy so the PSUM slot frees before the normalize chain
                    cs = stage_p.tile([128, S], F32, name="stg", tag="stg")
                    nc.vector.tensor_copy(cs[0:65, :], av[0:65, :])
                    rin = rin_p.tile([2, 1024], F32, name="rin", tag="rin")
                    nc.sync.dma_start(out=rin[0:1, :], in_=cs[64:65, 0:1024])
                    nc.sync.dma_start(out=rin[1:2, :], in_=cs[64:65, 1024:2048])
                    nc.scalar.activation(out=rin, in_=rin, func=Ln)
                    nc.scalar.activation(out=rin, in_=rin, func=Exp, scale=-1.0)
                    rd = dram_p.tile([2, 1024], F32, name="rd", tag="rd")
                    nc.sync.dma_start(out=rd, in_=rin)

                    # normalize + place into concat^T
                    for hh in range(2):
                        rb = rb_p.tile([64, 1024], F32, name="rb", tag="rb")
                        src = rd[hh, :]
                        bcast = bass.AP(tensor=src.tensor, offset=src.offset,
                                        ap=[[0, 64]] + list(src.ap))
                        nc.gpsimd.dma_start(out=rb, in_=bcast)
                        if hh == 0:
                            nc.vector.tensor_mul(
                                cat[p][0:64, 1024 * half:1024 * (half + 1)],
                                cs[0:64, 0:1024], rb)
                        else:
                            nt = ntmp_p.tile([64, 1024], DT, name="nt", tag="nt")
                            nc.vector.tensor_mul(nt, cs[0:64, 1024:2048], rb)
                            nc.gpsimd.dma_start(
                                out=cat[p][64:128, 1024 * half:1024 * (half + 1)],
                                in_=nt)

            # startup: q01+k01 chase the x^T stream together, then v01 + V'01;
            # pair-1 sets slot into attention-0's psB gaps (between halves)
            for _rep in range(repeat):
                wq01, dq01 = load_wset(0, 0)
                wk01, dk01 = load_wset(0, 1)
                xt = []
                for k in range(KT):
                    t = xt_p.tile([128, S], DT, name=f"xt{k}", tag="xt")
                    load_rounded(t, xT[128 * k:128 * (k + 1), :])
                    xt.append(t)
                wop = []
                for p in range(2):
                    t = wop_p.tile([128, D], DT, name=f"wop{p}", tag="wop")
                    load_rounded(t, wo[p, :, :])
                    wop.append(t)

                emit_qk_chase(0, wq01, dq01, wk01, dk01)
                emit_attention(
                    0,
                    prefix=lambda: (emit_qkv_set(0, 2, use_psB=True),
                                    emit_vprime_pair(0)),
                    half_hooks=(None,
                                lambda: emit_qkv_set(1, 0, use_psB=True)))
                emit_qkv_set(1, 1)
                emit_attention(
                    1,
                    prefix=lambda: (emit_qkv_set(1, 2, use_psB=True),
                                    emit_vprime_pair(1)))

                if dbg:
                    nc.sync.dma_start(out=dbg_t["xt0"][:, :],
                                      in_=xt[0].bitcast(F32))
                    nc.sync.dma_start(out=dbg_t["qt0"][:, :],
                                      in_=qt[0].bitcast(F32))
                    nc.sync.dma_start(out=dbg_t["kt0"][:, :],
                                      in_=kt[0].bitcast(F32))
                    nc.sync.dma_start(out=dbg_t["vt0"][:, :],
                                      in_=vt[0].bitcast(F32))
                    nc.sync.dma_start(
                        out=dbg_t["vp0"][:, 0:ST * 65],
                        in_=vpp[0].rearrange("p a b -> p (a b)").bitcast(F32))
                    nc.sync.dma_start(out=dbg_t["cat0"][:, :],
                                      in_=cat[0].bitcast(F32))
                # ---- output projection: accumulate over the 2 head pairs ----
                for st_i in range(ST):
                    fps = psA.tile([128, 1024], F32, name="fps", tag="psA")
                    for p in range(2):
                        for c in range(2):
                            nc.tensor.matmul(
                                fps[:, 512 * c:512 * (c + 1)],
                                cat[p][:, 128 * st_i:128 * (st_i + 1)],
                                wop[p][:, 512 * c:512 * (c + 1)],
                                start=(p == 0), stop=(p == 1))
                    fo = fo_p.tile([128, 1024], F32, name="fo", tag="fo")
                    nc.scalar.copy(fo, fps)
                    eng = nc.sync if st_i % 2 == 0 else nc.gpsimd
                    eng.dma_start(out=y[128 * st_i:128 * (st_i + 1), :], in_=fo)

    nc.finalize()
    return nc


def make_in_maps(x, Wq, Wk, Wv, bq, bk, bv, Wo):
    x = np.asarray(x, dtype=np.float32)
    Wq, Wk, Wv = (np.asarray(a, dtype=np.float32) for a in (Wq, Wk, Wv))
    bq, bk, bv = (np.asarray(a, dtype=np.float32) for a in (bq, bk, bv))
    Wo = np.asarray(Wo, dtype=np.float32)
    ident = np.eye(128, dtype=np.float32)
    xTb = [round_f32r(x[b].T) for b in range(B)]
    in_maps = []
    for core in range(NCORES):
        b, g = core // HG, core % HG
        wqkv = np.empty((2, 3, D, 128), dtype=np.float32)
        bqkv = np.empty((2, 3, 128), dtype=np.float32)
        wo_l = np.empty((2, 128, D), dtype=np.float32)
        for p in range(2):
            h0, h1 = 4 * g + 2 * p, 4 * g + 2 * p + 1
            for j, (W, bb) in enumerate(((Wq, bq), (Wk, bk), (Wv, bv))):
                wqkv[p, j, :, 0:64] = W[h0]
                wqkv[p, j, :, 64:128] = W[h1]
                bqkv[p, j, 0:64] = bb[h0]
                bqkv[p, j, 64:128] = bb[h1]
            wo_l[p] = Wo[h0 * DH:(h1 + 1) * DH, :]
        in_maps.append({"xT": xTb[b], "wqkv": round_f32r(wqkv),
                        "bqkv": bqkv, "wo": round_f32r(wo_l), "ident": ident})
    return in_maps


def kernel(x, Wq, Wk, Wv, bq, bk, bv, Wo, bo, **_ignored):
    if "nc" not in _CACHE:
        _CACHE["nc"] = build_nc()
    nc = _CACHE["nc"]
    in_maps = make_in_maps(x, Wq, Wk, Wv, bq, bk, bv, Wo)
    res = run_bass_kernel_spmd(nc, in_maps, core_ids=list(range(NCORES)))
    bo = np.asarray(bo, dtype=np.float32)
    out = np.zeros((B, S, D), dtype=np.float32)
    for core in range(NCORES):
        out[core // HG] += res.results[core]["y"]
    out += bo[None, None, :]
    return out
